# revision 14
# baseline (speedup 1.0000x reference)
"""Trainium2 Bass kernel for ConstantTimeStrideAttention.

Model (reference.py):
  qkv = x @ Wqkv + bqkv -> q,k,v per head (B=2, S=2048, DIM=1536, H=12, HD=128)
  per query s: 12 anchors (6 local +-1..3, 4 strided +-5,+-10, 2 global {0,S-1})
  attn = softmax(q . k_anchor * HD^-0.5 + log(group_weight)); out = attn @ v_anchors
  y = concat_heads @ Wout + bout

Sharding: 8 cores = (2 batches) x (4 sequence chunks of 512 queries). Each core
recomputes the k/v halo (+-10 tokens) and the two global tokens from the full x
input, so there are no collectives.

Device layout is feature-major ("transposed"): xT [DIM, 536 ext tokens] ->
qT/kT per head [128, toks]; V is produced token-major [toks, feats] so that the
attention AV matmul needs no transposes. Scores are computed transposed
(PT[k, q]) via kT-stationary matmuls; softmax runs as exp (ScalarE) ->
mask-multiply (VectorE; the mask carries the per-group softmax weights on the
anchor diagonals and zeroes everything else) -> denominator via an all-ones
stationary matmul that replicates the per-query sum across all 128 partitions
(so the reciprocal needs no partition broadcast) -> AV matmul -> multiply by
replicated reciprocal during PSUM evacuation. The per-token 1/denominator
commutes through the output projection, which consumes attn_T directly.

All matmuls run as float32r (FP22 multiply, fp32 accumulate): full PE speed for
moving dim >= 256, ~1e-4 relative error.
"""

import sys

sys.path.insert(0, "/opt/trn_rl_repo")

import numpy as np  # noqa: E402

import concourse.bass as bass  # noqa: E402,F401
import concourse.tile as tile  # noqa: E402
from concourse import bacc, mybir  # noqa: E402
from concourse import bass_utils  # noqa: E402

F32 = mybir.dt.float32
F32R = mybir.dt.float32r
EXP = mybir.ActivationFunctionType.Exp

B, S, DIM = 2, 2048, 1536
H, HD = 12, 128
SIGMA = 12
NCORES = 8
SCHUNKS = 4          # sequence chunks per batch
Q = S // SCHUNKS     # 512 queries per core
WIN = 10             # halo: max |offset|
EXT = 2 + (Q + 2 * WIN) + 2   # 536 ext k/v columns: [g0 g1][window 532][g0 g1]
NF = DIM // 128      # 12 contraction chunks
OFFS = [-3, -2, -1, 1, 2, 3, -10, -5, 5, 10]
# k-row chunks per query tile (256 queries each): (start, size) in ext cols
CHUNKS = [[(0, 128), (128, 128), (256, 128)],
          [(256, 128), (384, 128), (512, EXT - 512)]]

_CACHE = {}


def _build_program():
    nc = bacc.Bacc("TRN2", target_bir_lowering=False, debug=False)

    xt_d = nc.dram_tensor("xt", [DIM, EXT], F32R, kind="ExternalInput").ap()
    # weights pre-tiled on host: [group, fchunk, 128, 512] contiguous per tile
    wqkv_d = nc.dram_tensor("wqkv", [9, NF, 128, 512], F32R, kind="ExternalInput").ap()
    wout_d = nc.dram_tensor("wout", [3, NF, 128, 512], F32R, kind="ExternalInput").ap()
    # bias columns pre-transposed on host: [:, 0:24]=bq|bk per head, [:, 24:36]=bo
    bcol_d = nc.dram_tensor("bcol", [128, 3 * H], F32, kind="ExternalInput").ap()
    # V bias rows pre-broadcast on host to all 128 partitions: [128, 3*512]
    bvb_d = nc.dram_tensor("bvb", [128, 3 * 512], F32, kind="ExternalInput").ap()
    ones_d = nc.dram_tensor("ones_sq", [128, 128], F32R, kind="ExternalInput").ap()
    masks_d = nc.dram_tensor("masks", [6, 128, 256], F32, kind="ExternalInput").ap()
    yt_d = nc.dram_tensor("yt", [DIM, Q], F32, kind="ExternalOutput").ap()

    QCOL0 = 2 + WIN  # ext col of the first query token

    with tile.TileContext(nc) as tc:
        const = tc.alloc_tile_pool(name="const", bufs=1)
        at_pool = tc.alloc_tile_pool(name="at", bufs=1)
        xt_pool = tc.alloc_tile_pool(name="xt", bufs=1)
        wq_pool = tc.alloc_tile_pool(name="wq", bufs=3)
        qT_pool = tc.alloc_tile_pool(name="qT", bufs=6)
        kT_pool = tc.alloc_tile_pool(name="kT", bufs=6)
        v_pool = tc.alloc_tile_pool(name="v", bufs=10)
        et_pool = tc.alloc_tile_pool(name="et", bufs=3)
        ptm_pool = tc.alloc_tile_pool(name="ptm", bufs=3)
        rec_pool = tc.alloc_tile_pool(name="rec", bufs=2)
        yt_sb_pool = tc.alloc_tile_pool(name="yt_sb", bufs=2)

        q_ps = tc.alloc_tile_pool(name="q_ps", bufs=1, space="PSUM")
        k_ps = tc.alloc_tile_pool(name="k_ps", bufs=1, space="PSUM")
        v_ps = tc.alloc_tile_pool(name="v_ps", bufs=1, space="PSUM")
        pt_ps = tc.alloc_tile_pool(name="pt_ps", bufs=2, space="PSUM")
        av_ps = tc.alloc_tile_pool(name="av_ps", bufs=1, space="PSUM")
        dn_ps = tc.alloc_tile_pool(name="dn_ps", bufs=1, space="PSUM")
        yt_ps = tc.alloc_tile_pool(name="yt_ps", bufs=1, space="PSUM")

        # ---- x ext, transposed: one 3.3MB DMA on the sync HWDGE ring ----
        xt_t = xt_pool.tile([128, NF * EXT], F32R, tag="xt")
        for f in range(NF):
            nc.sync.dma_start(
                xt_t[:, f * EXT : (f + 1) * EXT], xt_d[f * 128 : (f + 1) * 128, :]
            )
        xt = [xt_t[:, f * EXT : (f + 1) * EXT] for f in range(NF)]

        # ---- constants (scalar HWDGE ring; emitted after the first weight
        # group so the startup-critical g6 tiles go first on that ring) ----
        def load_consts_small():
            ones_t = const.tile([128, 128], F32R, tag="ones")
            nc.scalar.dma_start(ones_t[:], ones_d[:])
            bcol_t = const.tile([128, 3 * H], F32, tag="bcol")
            nc.scalar.dma_start(bcol_t[:], bcol_d[:])
            bvb_full = const.tile([128, 3 * 512], F32, tag="bvb")
            nc.scalar.dma_start(bvb_full[:], bvb_d[:])
            bqk_t = [bcol_t[:, i : i + 1] for i in range(2 * H)]
            bo_t = [bcol_t[:, 2 * H + i : 2 * H + i + 1] for i in range(H)]
            bvb_t = [bvb_full[:, g * 512 : (g + 1) * 512] for g in range(3)]
            return ones_t, bqk_t, bo_t, bvb_t

        def load_masks():
            mask_t = []
            for i in range(6):
                m = const.tile([128, 256], F32, tag=f"mask{i}")
                nc.gpsimd.dma_start(m[:], masks_d[i])
                mask_t.append(m)
            return mask_t

        # token chunks of the ext axis (for token-major V)
        TCH = [(c * 128, min(128, EXT - c * 128)) for c in range((EXT + 127) // 128)]

        qT = [None] * H
        kT = [None] * H
        vv = [[None] * 3 for _ in TCH]
        at = [None] * H

        _wq_n = [0]

        def load_wq(g, src_d=wqkv_d):
            # One 3MB contiguous DMA per weight group. Ring assignment: the
            # startup-critical groups go on the two idle HWDGE rings (scalar:
            # g6; sync: g0 after xt), later groups stream on the gpsimd SWDGE
            # ring which runs concurrently.
            n = _wq_n[0]
            _wq_n[0] += 1
            t = wq_pool.tile([128, NF * 512], F32R, tag="wqg")
            # per-f slice DMAs: consuming matmuls unblock as each 256KB slice
            # lands instead of waiting for the whole 3MB group transfer
            eng = nc.scalar if n == 0 else (nc.sync if n == 1 else nc.gpsimd)
            for f in range(NF):
                eng.dma_start(t[:, f * 512 : (f + 1) * 512], src_d[g, f])
            return [t[:, f * 512 : (f + 1) * 512] for f in range(NF)]

        def emit_v_group(g, wt):
            for c, (cs, csz) in enumerate(TCH):
                ps = v_ps.tile([csz, 512], F32)
                for f in range(NF):
                    nc.tensor.matmul(
                        ps[:], xt[f][:, cs : cs + csz], wt[f][:],
                        start=(f == 0), stop=(f == NF - 1),
                    )
                sb = v_pool.tile([csz, 512], F32R, tag="v")
                nc.vector.tensor_add(sb[:], ps[:], bvb_t[g][:csz, :])
                vv[c][g] = sb

        def emit_q_chunk(hcur, wt):
            hx = hcur % 4
            ps = q_ps.tile([128, Q], F32)
            for f in range(NF):
                nc.tensor.matmul(
                    ps[:], wt[f][:, hx * 128 : (hx + 1) * 128],
                    xt[f][:, QCOL0 : QCOL0 + Q],
                    start=(f == 0), stop=(f == NF - 1),
                )
            sb = qT_pool.tile([128, Q], F32R, tag="qT")
            nc.vector.tensor_scalar_add(sb[:], ps[:], bqk_t[hcur][:])
            qT[hcur] = sb

        def emit_k_block(blk, wt):
            half = EXT // 2
            for hx in range(4):
                hcur = blk * 4 + hx
                sb = kT_pool.tile([128, EXT], F32R, tag="kT")
                for j in range(2):
                    ps = k_ps.tile([128, half], F32)
                    for f in range(NF):
                        nc.tensor.matmul(
                            ps[:], wt[f][:, hx * 128 : (hx + 1) * 128],
                            xt[f][:, j * half : (j + 1) * half],
                            start=(f == 0), stop=(f == NF - 1),
                        )
                    nc.vector.tensor_scalar_add(
                        sb[:, j * half : (j + 1) * half], ps[:], bqk_t[H + hcur][:]
                    )
                kT[hcur] = sb

        def emit_attention(h):
            sb = at_pool.tile([128, Q], F32R, tag=f"at{h}")
            for t in range(2):
                avp = av_ps.tile([128, 256], F32)
                dnp = dn_ps.tile([128, 256], F32)
                nch = len(CHUNKS[t])
                for ci, (cs, csz) in enumerate(CHUNKS[t]):
                    ptp = pt_ps.tile([csz, 256], F32)
                    nc.tensor.matmul(
                        ptp[:], kT[h][:, cs : cs + csz],
                        qT[h][:, t * 256 : (t + 1) * 256],
                        start=True, stop=True,
                    )
                    et = et_pool.tile([csz, 256], F32, tag="et")
                    nc.scalar.activation(et[:], ptp[:], EXP)
                    ptm = ptm_pool.tile([csz, 256], F32R, tag="ptm")
                    nc.vector.tensor_mul(ptm[:], et[:], mask_t[t * 3 + ci][:csz, :])
                    nc.tensor.matmul(
                        avp[:], vv[cs // 128][h // 4][:csz, (h % 4) * 128 : (h % 4 + 1) * 128],
                        ptm[:], start=(ci == 0), stop=(ci == nch - 1),
                    )
                    nc.tensor.matmul(
                        dnp[:], ones_t[:csz, :], ptm[:],
                        start=(ci == 0), stop=(ci == nch - 1),
                    )
                rec = rec_pool.tile([128, 256], F32, tag="rec")
                nc.vector.reciprocal_approx_fast(rec[:], dnp[:])
                nc.vector.tensor_mul(sb[:, t * 256 : (t + 1) * 256], avp[:], rec[:])
            at[h] = sb

        # ---- emission order: V group then Q,K then attention, per head block.
        # consts are emitted right after the first weight-group DMAs so the
        # scalar ring services g6 first.
        ones_t = mask_t = bqk_t = bo_t = bvb_t = None
        for blk in range(3):
            wt_v = load_wq(6 + blk)               # blk0: scalar ring, sliced
            if ones_t is None:
                ones_t, bqk_t, bo_t, bvb_t = load_consts_small()
            emit_v_group(blk, wt_v)
            wt_k = load_wq(3 + blk)               # blk0: sync ring after xt
            emit_k_block(blk, wt_k)
            wt_q = load_wq(blk)                   # gpsimd SWDGE ring
            if mask_t is None:
                mask_t = load_masks()
            for h in range(blk * 4, blk * 4 + 4):
                emit_q_chunk(h, wt_q)
                emit_attention(h)

        for og in range(3):
            wt = load_wq(og, src_d=wout_d)
            for oc in range(4):
                o = og * 4 + oc
                ps = yt_ps.tile([128, Q], F32)
                for f in range(NF):
                    nc.tensor.matmul(
                        ps[:], wt[f][:, oc * 128 : (oc + 1) * 128], at[f][:],
                        start=(f == 0), stop=(f == NF - 1),
                    )
                sb = yt_sb_pool.tile([128, Q], F32, tag="yt")
                nc.vector.tensor_scalar_add(sb[:], ps[:], bo_t[o][:])
                nc.sync.dma_start(yt_d[o * 128 : (o + 1) * 128, :], sb[:])

        for p in (yt_ps, dn_ps, av_ps, pt_ps, v_ps, k_ps, q_ps):
            p.release()
        for p in (yt_sb_pool, rec_pool, ptm_pool, et_pool, v_pool, kT_pool,
                  qT_pool, wq_pool, xt_pool, at_pool, const):
            p.release()

    nc.compile()
    return nc


def _softmax(v):
    e = np.exp(v - v.max())
    return e / e.sum()


def _build_masks(r0, gw):
    """Per-core mask tiles [6, 128, 256] routing softmax group weights onto the
    anchor positions of the transposed score chunks."""
    lo = r0 - WIN
    masks = np.zeros((6, 128, 256), np.float32)
    wts = [gw[0]] * 6 + [gw[1]] * 4
    for qi in range(Q):
        t, qq = divmod(qi, 256)

        def add(col, w):
            for ci, (cs, csz) in enumerate(CHUNKS[t]):
                if cs <= col < cs + csz:
                    masks[t * 3 + ci, col - cs, qq] += w
                    return
            raise AssertionError(f"col {col} not covered for qtile {t}")

        for off, w in zip(OFFS, wts):
            tok = min(max(r0 + qi + off, 0), S - 1)
            add(2 + (tok - lo), w)
        # global anchors: duplicated at both ends of the ext axis
        add(0 if t == 0 else EXT - 2, gw[2])   # token 0
        add(1 if t == 0 else EXT - 1, gw[2])   # token S-1
    return masks


def _prepare_in_maps(x, wqkv, bqkv, wout, bout, group_scale):
    scale = HD ** -0.5
    wqkv_m = np.array(wqkv, np.float32, copy=True)
    wqkv_m[:, :DIM] *= scale
    # pre-tile: [9 groups, 12 fchunks, 128, 512] contiguous per [128,512] tile
    wqkv_t = np.ascontiguousarray(
        wqkv_m.reshape(NF, 128, 9, 512).transpose(2, 0, 1, 3)
    )
    bqkv_m = np.array(bqkv, np.float32, copy=True)
    bqkv_m[:DIM] *= scale
    gw = _softmax(np.asarray(group_scale, np.float64))

    # bias columns [128, 36]: q heads, k heads, then out-proj chunks
    bcol = np.concatenate(
        [
            bqkv_m[:DIM].reshape(H, 128),
            bqkv_m[DIM : 2 * DIM].reshape(H, 128),
            np.asarray(bout, np.float32).reshape(H, 128),
        ],
        axis=0,
    ).T.astype(np.float32).copy()  # [128, 36]
    bvb = np.broadcast_to(bqkv_m[2 * DIM :][None, :], (128, 3 * 512)).astype(
        np.float32
    ).copy()
    wout_t = np.ascontiguousarray(
        np.asarray(wout, np.float32).reshape(NF, 128, 3, 512).transpose(2, 0, 1, 3)
    )
    ones_sq = np.ones((128, 128), np.float32)

    in_maps = []
    for core in range(NCORES):
        b, sc = divmod(core, SCHUNKS)
        r0 = sc * Q
        lo = r0 - WIN
        tok_ids = np.concatenate(
            [
                [0, S - 1],
                np.clip(np.arange(lo, lo + Q + 2 * WIN), 0, S - 1),
                [0, S - 1],
            ]
        ).astype(np.int64)
        x_ext_t = np.ascontiguousarray(x[b, tok_ids, :].T)  # [DIM, EXT]
        masks = _build_masks(r0, gw)
        in_maps.append(
            {
                "xt": x_ext_t,
                "wqkv": wqkv_t,
                "wout": wout_t,
                "bcol": bcol,
                "bvb": bvb,
                "ones_sq": ones_sq,
                "masks": masks,
            }
        )
    return in_maps


def get_program():
    if "nc" not in _CACHE:
        _CACHE["nc"] = _build_program()
    return _CACHE["nc"]


def run(inputs, **spmd_kwargs):
    """Run the SPMD kernel; returns (y [B,S,DIM] fp32, BassKernelResults)."""
    x = np.asarray(inputs["x"], np.float32)
    in_maps = _prepare_in_maps(
        x,
        np.asarray(inputs["Wqkv"], np.float32),
        np.asarray(inputs["bqkv"], np.float32),
        np.asarray(inputs["Wout"], np.float32),
        np.asarray(inputs["bout"], np.float32),
        np.asarray(inputs["group_scale"], np.float32),
    )
    nc = get_program()
    res = bass_utils.run_bass_kernel_spmd(
        nc, in_maps, core_ids=list(range(NCORES)), **spmd_kwargs
    )
    y = np.empty((B, S, DIM), np.float32)
    for core in range(NCORES):
        b, sc = divmod(core, SCHUNKS)
        y[b, sc * Q : (sc + 1) * Q, :] = res.results[core]["yt"].T
    return y, res


def kernel(**inputs):
    y, _ = run(inputs)
    return y


# revision 15
# speedup vs baseline: 1.0017x; 1.0017x over previous
"""Trainium2 Bass kernel for ConstantTimeStrideAttention.

Model (reference.py):
  qkv = x @ Wqkv + bqkv -> q,k,v per head (B=2, S=2048, DIM=1536, H=12, HD=128)
  per query s: 12 anchors (6 local +-1..3, 4 strided +-5,+-10, 2 global {0,S-1})
  attn = softmax(q . k_anchor * HD^-0.5 + log(group_weight)); out = attn @ v_anchors
  y = concat_heads @ Wout + bout

Sharding: 8 cores = (2 batches) x (4 sequence chunks of 512 queries). Each core
recomputes the k/v halo (+-10 tokens) and the two global tokens from the full x
input, so there are no collectives.

Device layout is feature-major ("transposed"): xT [DIM, 536 ext tokens] ->
qT/kT per head [128, toks]; V is produced token-major [toks, feats] so that the
attention AV matmul needs no transposes. Scores are computed transposed
(PT[k, q]) via kT-stationary matmuls; softmax runs as exp (ScalarE) ->
mask-multiply (VectorE; the mask carries the per-group softmax weights on the
anchor diagonals and zeroes everything else) -> denominator via an all-ones
stationary matmul that replicates the per-query sum across all 128 partitions
(so the reciprocal needs no partition broadcast) -> AV matmul -> multiply by
replicated reciprocal during PSUM evacuation. The per-token 1/denominator
commutes through the output projection, which consumes attn_T directly.

All matmuls run as float32r (FP22 multiply, fp32 accumulate): full PE speed for
moving dim >= 256, ~1e-4 relative error.
"""

import sys

sys.path.insert(0, "/opt/trn_rl_repo")

import numpy as np  # noqa: E402

import concourse.bass as bass  # noqa: E402,F401
import concourse.tile as tile  # noqa: E402
from concourse import bacc, mybir  # noqa: E402
from concourse import bass_utils  # noqa: E402

F32 = mybir.dt.float32
F32R = mybir.dt.float32r
EXP = mybir.ActivationFunctionType.Exp

B, S, DIM = 2, 2048, 1536
H, HD = 12, 128
SIGMA = 12
NCORES = 8
SCHUNKS = 4          # sequence chunks per batch
Q = S // SCHUNKS     # 512 queries per core
WIN = 10             # halo: max |offset|
EXT = 2 + (Q + 2 * WIN) + 2   # 536 ext k/v columns: [g0 g1][window 532][g0 g1]
NF = DIM // 128      # 12 contraction chunks
OFFS = [-3, -2, -1, 1, 2, 3, -10, -5, 5, 10]
# k-row chunks per query tile (256 queries each): (start, size) in ext cols
CHUNKS = [[(0, 128), (128, 128), (256, 128)],
          [(256, 128), (384, 128), (512, EXT - 512)]]

_CACHE = {}


def _build_program():
    nc = bacc.Bacc("TRN2", target_bir_lowering=False, debug=False)

    xt_d = nc.dram_tensor("xt", [DIM, EXT], F32R, kind="ExternalInput").ap()
    # weights pre-tiled on host: [group, fchunk, 128, 512] contiguous per tile
    wqkv_d = nc.dram_tensor("wqkv", [9, NF, 128, 512], F32R, kind="ExternalInput").ap()
    wout_d = nc.dram_tensor("wout", [3, NF, 128, 512], F32R, kind="ExternalInput").ap()
    # bias columns pre-transposed on host: [:, 0:24]=bq|bk per head, [:, 24:36]=bo
    bcol_d = nc.dram_tensor("bcol", [128, 3 * H], F32, kind="ExternalInput").ap()
    # V bias rows pre-broadcast on host to all 128 partitions: [128, 3*512]
    bvb_d = nc.dram_tensor("bvb", [128, 3 * 512], F32, kind="ExternalInput").ap()
    ones_d = nc.dram_tensor("ones_sq", [128, 128], F32R, kind="ExternalInput").ap()
    masks_d = nc.dram_tensor("masks", [6, 128, 256], F32, kind="ExternalInput").ap()
    yt_d = nc.dram_tensor("yt", [DIM, Q], F32, kind="ExternalOutput").ap()

    QCOL0 = 2 + WIN  # ext col of the first query token

    with tile.TileContext(nc) as tc:
        const = tc.alloc_tile_pool(name="const", bufs=1)
        at_pool = tc.alloc_tile_pool(name="at", bufs=1)
        xt_pool = tc.alloc_tile_pool(name="xt", bufs=1)
        wq_pool = tc.alloc_tile_pool(name="wq", bufs=3)
        qT_pool = tc.alloc_tile_pool(name="qT", bufs=6)
        kT_pool = tc.alloc_tile_pool(name="kT", bufs=6)
        v_pool = tc.alloc_tile_pool(name="v", bufs=10)
        et_pool = tc.alloc_tile_pool(name="et", bufs=3)
        ptm_pool = tc.alloc_tile_pool(name="ptm", bufs=3)
        rec_pool = tc.alloc_tile_pool(name="rec", bufs=2)
        yt_sb_pool = tc.alloc_tile_pool(name="yt_sb", bufs=2)

        q_ps = tc.alloc_tile_pool(name="q_ps", bufs=1, space="PSUM")
        k_ps = tc.alloc_tile_pool(name="k_ps", bufs=2, space="PSUM")
        pt_ps = tc.alloc_tile_pool(name="pt_ps", bufs=2, space="PSUM")
        av_ps = tc.alloc_tile_pool(name="av_ps", bufs=1, space="PSUM")
        dn_ps = tc.alloc_tile_pool(name="dn_ps", bufs=1, space="PSUM")
        # allocated last in the PSUM stack: released after the final V group so
        # its bank can host yt_ps, letting the output projection overlap the
        # attention tail instead of waiting on all stage-PSUM users
        v_ps = tc.alloc_tile_pool(name="v_ps", bufs=1, space="PSUM")
        yt_ps_holder = [None]

        # ---- x ext, transposed: one 3.3MB DMA on the sync HWDGE ring ----
        xt_t = xt_pool.tile([128, NF * EXT], F32R, tag="xt")
        for f in range(NF):
            nc.sync.dma_start(
                xt_t[:, f * EXT : (f + 1) * EXT], xt_d[f * 128 : (f + 1) * 128, :]
            )
        xt = [xt_t[:, f * EXT : (f + 1) * EXT] for f in range(NF)]

        # ---- constants (scalar HWDGE ring; emitted after the first weight
        # group so the startup-critical g6 tiles go first on that ring) ----
        def load_consts_small():
            ones_t = const.tile([128, 128], F32R, tag="ones")
            nc.scalar.dma_start(ones_t[:], ones_d[:])
            bcol_t = const.tile([128, 3 * H], F32, tag="bcol")
            nc.scalar.dma_start(bcol_t[:], bcol_d[:])
            bvb_full = const.tile([128, 3 * 512], F32, tag="bvb")
            nc.scalar.dma_start(bvb_full[:], bvb_d[:])
            bqk_t = [bcol_t[:, i : i + 1] for i in range(2 * H)]
            bo_t = [bcol_t[:, 2 * H + i : 2 * H + i + 1] for i in range(H)]
            bvb_t = [bvb_full[:, g * 512 : (g + 1) * 512] for g in range(3)]
            return ones_t, bqk_t, bo_t, bvb_t

        def load_masks():
            mask_t = []
            for i in range(6):
                m = const.tile([128, 256], F32, tag=f"mask{i}")
                nc.gpsimd.dma_start(m[:], masks_d[i])
                mask_t.append(m)
            return mask_t

        # token chunks of the ext axis (for token-major V)
        TCH = [(c * 128, min(128, EXT - c * 128)) for c in range((EXT + 127) // 128)]

        qT = [None] * H
        kT = [None] * H
        vv = [[None] * 3 for _ in TCH]
        at = [None] * H

        _wq_n = [0]

        def load_wq(g, src_d=wqkv_d):
            # One 3MB contiguous DMA per weight group. Ring assignment: the
            # startup-critical groups go on the two idle HWDGE rings (scalar:
            # g6; sync: g0 after xt), later groups stream on the gpsimd SWDGE
            # ring which runs concurrently.
            n = _wq_n[0]
            _wq_n[0] += 1
            t = wq_pool.tile([128, NF * 512], F32R, tag="wqg")
            # per-f slice DMAs: consuming matmuls unblock as each 256KB slice
            # lands instead of waiting for the whole 3MB group transfer
            eng = nc.scalar if n == 0 else (nc.sync if n == 1 else nc.gpsimd)
            for f in range(NF):
                eng.dma_start(t[:, f * 512 : (f + 1) * 512], src_d[g, f])
            return [t[:, f * 512 : (f + 1) * 512] for f in range(NF)]

        def emit_v_group(g, wt):
            for c, (cs, csz) in enumerate(TCH):
                ps = v_ps.tile([csz, 512], F32)
                for f in range(NF):
                    nc.tensor.matmul(
                        ps[:], xt[f][:, cs : cs + csz], wt[f][:],
                        start=(f == 0), stop=(f == NF - 1),
                    )
                sb = v_pool.tile([csz, 512], F32R, tag="v")
                nc.vector.tensor_add(sb[:], ps[:], bvb_t[g][:csz, :])
                vv[c][g] = sb

        def emit_q_chunk(hcur, wt):
            hx = hcur % 4
            ps = q_ps.tile([128, Q], F32)
            for f in range(NF):
                nc.tensor.matmul(
                    ps[:], wt[f][:, hx * 128 : (hx + 1) * 128],
                    xt[f][:, QCOL0 : QCOL0 + Q],
                    start=(f == 0), stop=(f == NF - 1),
                )
            sb = qT_pool.tile([128, Q], F32R, tag="qT")
            nc.vector.tensor_scalar_add(sb[:], ps[:], bqk_t[hcur][:])
            qT[hcur] = sb

        def emit_k_block(blk, wt):
            half = EXT // 2
            for hx in range(4):
                hcur = blk * 4 + hx
                sb = kT_pool.tile([128, EXT], F32R, tag="kT")
                for j in range(2):
                    ps = k_ps.tile([128, half], F32)
                    for f in range(NF):
                        nc.tensor.matmul(
                            ps[:], wt[f][:, hx * 128 : (hx + 1) * 128],
                            xt[f][:, j * half : (j + 1) * half],
                            start=(f == 0), stop=(f == NF - 1),
                        )
                    nc.vector.tensor_scalar_add(
                        sb[:, j * half : (j + 1) * half], ps[:], bqk_t[H + hcur][:]
                    )
                kT[hcur] = sb

        def emit_attention(h):
            sb = at_pool.tile([128, Q], F32R, tag=f"at{h}")
            for t in range(2):
                avp = av_ps.tile([128, 256], F32)
                dnp = dn_ps.tile([128, 256], F32)
                nch = len(CHUNKS[t])
                for ci, (cs, csz) in enumerate(CHUNKS[t]):
                    ptp = pt_ps.tile([csz, 256], F32)
                    nc.tensor.matmul(
                        ptp[:], kT[h][:, cs : cs + csz],
                        qT[h][:, t * 256 : (t + 1) * 256],
                        start=True, stop=True,
                    )
                    et = et_pool.tile([csz, 256], F32, tag="et")
                    nc.scalar.activation(et[:], ptp[:], EXP)
                    ptm = ptm_pool.tile([csz, 256], F32R, tag="ptm")
                    nc.vector.tensor_mul(ptm[:], et[:], mask_t[t * 3 + ci][:csz, :])
                    nc.tensor.matmul(
                        avp[:], vv[cs // 128][h // 4][:csz, (h % 4) * 128 : (h % 4 + 1) * 128],
                        ptm[:], start=(ci == 0), stop=(ci == nch - 1),
                    )
                    nc.tensor.matmul(
                        dnp[:], ones_t[:csz, :], ptm[:],
                        start=(ci == 0), stop=(ci == nch - 1),
                    )
                rec = rec_pool.tile([128, 256], F32, tag="rec")
                nc.vector.reciprocal_approx_fast(rec[:], dnp[:])
                nc.vector.tensor_mul(sb[:, t * 256 : (t + 1) * 256], avp[:], rec[:])
            at[h] = sb

        # ---- emission order: V group then Q,K then attention, per head block.
        # consts are emitted right after the first weight-group DMAs so the
        # scalar ring services g6 first.
        ones_t = mask_t = bqk_t = bo_t = bvb_t = None
        for blk in range(3):
            wt_v = load_wq(6 + blk)               # blk0: scalar ring, sliced
            if ones_t is None:
                ones_t, bqk_t, bo_t, bvb_t = load_consts_small()
            emit_v_group(blk, wt_v)
            if blk == 2:
                v_ps.release()
                yt_ps_holder[0] = tc.alloc_tile_pool(name="yt_ps", bufs=1, space="PSUM")
            wt_k = load_wq(3 + blk)               # blk0: sync ring after xt
            emit_k_block(blk, wt_k)
            wt_q = load_wq(blk)                   # gpsimd SWDGE ring
            if mask_t is None:
                mask_t = load_masks()
            for h in range(blk * 4, blk * 4 + 4):
                emit_q_chunk(h, wt_q)
                emit_attention(h)

        yt_ps = yt_ps_holder[0]
        for og in range(3):
            wt = load_wq(og, src_d=wout_d)
            for oc in range(4):
                o = og * 4 + oc
                ps = yt_ps.tile([128, Q], F32)
                for f in range(NF):
                    nc.tensor.matmul(
                        ps[:], wt[f][:, oc * 128 : (oc + 1) * 128], at[f][:],
                        start=(f == 0), stop=(f == NF - 1),
                    )
                sb = yt_sb_pool.tile([128, Q], F32, tag="yt")
                nc.vector.tensor_scalar_add(sb[:], ps[:], bo_t[o][:])
                nc.sync.dma_start(yt_d[o * 128 : (o + 1) * 128, :], sb[:])

        for p in (yt_ps, dn_ps, av_ps, pt_ps, k_ps, q_ps):
            p.release()
        for p in (yt_sb_pool, rec_pool, ptm_pool, et_pool, v_pool, kT_pool,
                  qT_pool, wq_pool, xt_pool, at_pool, const):
            p.release()

    nc.compile()
    return nc


def _softmax(v):
    e = np.exp(v - v.max())
    return e / e.sum()


def _build_masks(r0, gw):
    """Per-core mask tiles [6, 128, 256] routing softmax group weights onto the
    anchor positions of the transposed score chunks."""
    lo = r0 - WIN
    masks = np.zeros((6, 128, 256), np.float32)
    wts = [gw[0]] * 6 + [gw[1]] * 4
    for qi in range(Q):
        t, qq = divmod(qi, 256)

        def add(col, w):
            for ci, (cs, csz) in enumerate(CHUNKS[t]):
                if cs <= col < cs + csz:
                    masks[t * 3 + ci, col - cs, qq] += w
                    return
            raise AssertionError(f"col {col} not covered for qtile {t}")

        for off, w in zip(OFFS, wts):
            tok = min(max(r0 + qi + off, 0), S - 1)
            add(2 + (tok - lo), w)
        # global anchors: duplicated at both ends of the ext axis
        add(0 if t == 0 else EXT - 2, gw[2])   # token 0
        add(1 if t == 0 else EXT - 1, gw[2])   # token S-1
    return masks


def _prepare_in_maps(x, wqkv, bqkv, wout, bout, group_scale):
    scale = HD ** -0.5
    wqkv_m = np.array(wqkv, np.float32, copy=True)
    wqkv_m[:, :DIM] *= scale
    # pre-tile: [9 groups, 12 fchunks, 128, 512] contiguous per [128,512] tile
    wqkv_t = np.ascontiguousarray(
        wqkv_m.reshape(NF, 128, 9, 512).transpose(2, 0, 1, 3)
    )
    bqkv_m = np.array(bqkv, np.float32, copy=True)
    bqkv_m[:DIM] *= scale
    gw = _softmax(np.asarray(group_scale, np.float64))

    # bias columns [128, 36]: q heads, k heads, then out-proj chunks
    bcol = np.concatenate(
        [
            bqkv_m[:DIM].reshape(H, 128),
            bqkv_m[DIM : 2 * DIM].reshape(H, 128),
            np.asarray(bout, np.float32).reshape(H, 128),
        ],
        axis=0,
    ).T.astype(np.float32).copy()  # [128, 36]
    bvb = np.broadcast_to(bqkv_m[2 * DIM :][None, :], (128, 3 * 512)).astype(
        np.float32
    ).copy()
    wout_t = np.ascontiguousarray(
        np.asarray(wout, np.float32).reshape(NF, 128, 3, 512).transpose(2, 0, 1, 3)
    )
    ones_sq = np.ones((128, 128), np.float32)

    in_maps = []
    for core in range(NCORES):
        b, sc = divmod(core, SCHUNKS)
        r0 = sc * Q
        lo = r0 - WIN
        tok_ids = np.concatenate(
            [
                [0, S - 1],
                np.clip(np.arange(lo, lo + Q + 2 * WIN), 0, S - 1),
                [0, S - 1],
            ]
        ).astype(np.int64)
        x_ext_t = np.ascontiguousarray(x[b, tok_ids, :].T)  # [DIM, EXT]
        masks = _build_masks(r0, gw)
        in_maps.append(
            {
                "xt": x_ext_t,
                "wqkv": wqkv_t,
                "wout": wout_t,
                "bcol": bcol,
                "bvb": bvb,
                "ones_sq": ones_sq,
                "masks": masks,
            }
        )
    return in_maps


def get_program():
    if "nc" not in _CACHE:
        _CACHE["nc"] = _build_program()
    return _CACHE["nc"]


def run(inputs, **spmd_kwargs):
    """Run the SPMD kernel; returns (y [B,S,DIM] fp32, BassKernelResults)."""
    x = np.asarray(inputs["x"], np.float32)
    in_maps = _prepare_in_maps(
        x,
        np.asarray(inputs["Wqkv"], np.float32),
        np.asarray(inputs["bqkv"], np.float32),
        np.asarray(inputs["Wout"], np.float32),
        np.asarray(inputs["bout"], np.float32),
        np.asarray(inputs["group_scale"], np.float32),
    )
    nc = get_program()
    res = bass_utils.run_bass_kernel_spmd(
        nc, in_maps, core_ids=list(range(NCORES)), **spmd_kwargs
    )
    y = np.empty((B, S, DIM), np.float32)
    for core in range(NCORES):
        b, sc = divmod(core, SCHUNKS)
        y[b, sc * Q : (sc + 1) * Q, :] = res.results[core]["yt"].T
    return y, res


def kernel(**inputs):
    y, _ = run(inputs)
    return y
